# revision 41
# baseline (speedup 1.0000x reference)
"""Trainium2 Bass kernel for ragged masked attention-score softmax.

Problem (B=32, T=8192, H=128):
    energy[b,t] = relu(W1 @ hidden[b] + W2 @ enc[t,b] + b_attn)   (W_attn = [W1 | W2])
    scores[b,t] = v . energy[b,t]
    out[b,0,:]  = ragged-masked softmax over t < len_seq[b], zeros after.

Strategy (8 NeuronCores, data-parallel over B):
  - Rows are sorted by len desc; slot j on every core takes one row from rank
    group [8j, 8j+8).  The per-slot static position count NP_j = max len in the
    group (rounded to 128), so all cores run one shared graph while skipping
    ~half the positions (ragged lengths are known at trace time).
  - Host passes each core's rows TRANSPOSED ([H, NP_j], H on partitions) so the
    device streams contiguous tiles straight into the TensorEngine with no
    on-device transpose of the big tensor.
  - Per 512-column tile: energy = W2T.T @ encT (PE) -> bias+relu alternating
    between ScalarE and VectorE (PSUM->SBUF, bf16) -> v-dot via PE (energy
    tile as stationary, v as moving operand) accumulating scores[t,1] columns
    into a per-slot PSUM scores tile.  V-dot emission lags one group so the
    PE never stalls on the relu engines.
  - Per-slot fused masked softmax overlaps the next slot's hot loop: exact max
    via PE transpose + ones-matmul broadcast, exp with fused -max bias, mask
    multiply, ones-matmul partition sums, and a PE transpose to t-major with
    the 1/sum normalize folded into the PSUM drain; then one DMA per slot.
  - Host-side prep is layout only: per-core row slices transposed to [H, NP_j]
    (+ bf16 cast), packed constant blocks, masks from len_seq, and the final
    gather of per-core outputs into [B, 1, T].
"""

from contextlib import ExitStack

import numpy as np

import concourse.bass as bass
import concourse.tile as tile
from concourse import bacc, mybir
from concourse.bass_utils import run_bass_kernel_spmd

B, T, H = 32, 8192, 128
NCORES = 8
SLOTS = B // NCORES  # 4 rows per core
NEG = -1.0e30

# knobs
COMPUTE_DTYPE = "bfloat16"  # "float32" | "bfloat16" for enc/W2/v/energy matmul path
CHUNK = 8192  # positions per enc chunk tile
LEAD = 512  # first slice of each chunk DMA'd separately so compute starts early
GRP = 512  # relu granularity (1 PSUM bank)
MMN = 512  # matmul moving free dim (one PSUM bank in f32)


def _np_dt(name):
    if name == "bfloat16":
        import ml_dtypes

        return np.dtype(ml_dtypes.bfloat16)
    return np.dtype(np.float32)


def _my_dt(name):
    return mybir.dt.bfloat16 if name == "bfloat16" else mybir.dt.float32


def _plan(ls, t_max):
    """Assign rows to (core, slot). Returns rows[core][slot] = b, NP[slot]."""
    order = np.argsort(-np.asarray(ls), kind="stable")
    rows = [[int(order[8 * j + i]) for j in range(SLOTS)] for i in range(NCORES)]
    NP = []
    for j in range(SLOTS):
        mx = int(max(ls[int(order[8 * j + i])] for i in range(NCORES)))
        NP.append(min(((mx + 127) // 128) * 128, t_max))
    return rows, NP


def _build(nc, NP, nt_out, dt_name):
    """Emit the Tile graph. NP: per-slot position counts (mult of 128).
    nt_out: number of 128-wide t-tiles in the output (T/128)."""
    dt_c = _my_dt(dt_name)
    f32 = mybir.dt.float32
    AF = mybir.ActivationFunctionType

    encs = [
        nc.dram_tensor(f"enc{j}", [H, NP[j]], dt_c, kind="ExternalInput").ap()
        for j in range(SLOTS)
    ]
    # packed consts: one f32 DMA + one compute-dtype DMA
    # constsf layout: [w1t(128) | bvec(1) | hid(4) | ident(128) | maskt(4*nt)]
    ncf = H + 1 + SLOTS + 128 + SLOTS * nt_out
    constsf = nc.dram_tensor("constsf", [128, ncf], f32, kind="ExternalInput").ap()
    consts16 = nc.dram_tensor("consts16", [128, H + 1], dt_c, kind="ExternalInput").ap()
    out = nc.dram_tensor("out", [SLOTS, nt_out, 128], f32, kind="ExternalOutput").ap()

    with ExitStack() as ctx:
        tc = ctx.enter_context(tile.TileContext(nc))
        singles = ctx.enter_context(tc.tile_pool(name="singles", bufs=1))
        encpool = ctx.enter_context(tc.tile_pool(name="encp", bufs=4))
        enpool = ctx.enter_context(tc.tile_pool(name="energy", bufs=4))
        smallp = ctx.enter_context(tc.tile_pool(name="small", bufs=3))
        outp = ctx.enter_context(tc.tile_pool(name="outp", bufs=2))
        ps_e = ctx.enter_context(tc.tile_pool(name="ps_e", bufs=3, space="PSUM"))
        ps_sc = ctx.enter_context(tc.tile_pool(name="ps_sc", bufs=3, space="PSUM"))
        ps_h = ctx.enter_context(tc.tile_pool(name="ps_h", bufs=1, space="PSUM"))
        ps_o = ctx.enter_context(tc.tile_pool(name="ps_o", bufs=1, space="PSUM"))

        # PE warm-up: dense dummy matmuls during the DMA-wait window release
        # the HAM clock gate (1.2 -> 2.4 GHz) before the real stream begins.
        dum = singles.tile([H, H], dt_c)
        nc.vector.memset(dum[:], 0.0)
        dume = singles.tile([1, 1], f32)
        nc.vector.memset(dume[:], 0.0)
        pdum = ps_h.tile([H, 1], f32, tag="ps_small")
        for _ in range(6):
            nc.tensor.matmul(
                out=pdum[:], lhsT=dum[:], rhs=dum[:, :1], start=True, stop=True
            )
        # preload the exp ACT table set while DMAs stream
        exp_warm = singles.tile([1, 1], f32)
        nc.scalar.activation(exp_warm[:], dume[:], AF.Exp)

        # DMA order: packed consts + a small lead slice of enc first so the
        # first energy matmul starts early; each chunk's remainder follows.
        c16_sb = singles.tile([128, H + 1], dt_c)
        nc.sync.dma_start(c16_sb[:], consts16[:])
        w2t_sb = c16_sb[:, :H]
        vvec_sb = c16_sb[:, H : H + 1]

        echunks = {}
        et0 = encpool.tile([H, CHUNK], dt_c, tag="enc")
        cw0 = min(CHUNK, NP[0])
        lead0 = min(LEAD, cw0)
        nc.sync.dma_start(et0[:, :lead0], encs[0][:, :lead0])
        echunks[(0, 0)] = et0

        cf_sb = singles.tile([128, ncf], f32)
        nc.sync.dma_start(cf_sb[:], constsf[:])
        w1t_sb = cf_sb[:, :H]
        bvec_sb = cf_sb[:, H : H + 1]
        hid_sb = cf_sb[:, H + 1 : H + 1 + SLOTS]
        ident_sb = cf_sb[:, H + 1 + SLOTS : H + 1 + SLOTS + 128]
        maskt_sb = cf_sb[:, H + 1 + SLOTS + 128 :].rearrange(
            "p (j t) -> p j t", j=SLOTS
        )

        if lead0 < cw0:
            nc.sync.dma_start(et0[:, lead0:cw0], encs[0][:, lead0:cw0])
        for j in range(SLOTS):
            for c0 in range(0, NP[j], CHUNK):
                if (j, c0) in echunks:
                    continue
                cw = min(CHUNK, NP[j] - c0)
                et = encpool.tile([H, CHUNK], dt_c, tag="enc")
                lead = min(LEAD, cw)
                nc.sync.dma_start(et[:, :lead], encs[j][:, c0 : c0 + lead])
                if lead < cw:
                    nc.sync.dma_start(
                        et[:, lead:cw], encs[j][:, c0 + lead : c0 + cw]
                    )
                echunks[(j, c0)] = et

        ones1 = singles.tile([1, 128], f32)
        nc.vector.memset(ones1[:], 1.0)
        ones_col = singles.tile([128, 1], f32)
        nc.vector.memset(ones_col[:], 1.0)

        # hproj[h,j] = W1 @ hidden_j + b
        ph = ps_h.tile([H, SLOTS], f32, tag="ps_small")
        nc.tensor.matmul(out=ph[:], lhsT=w1t_sb, rhs=hid_sb, start=True, stop=True)
        hproj = singles.tile([H, SLOTS], f32)
        nc.scalar.activation(hproj[:], ph[:], AF.Identity, bias=bvec_sb)

        # ---- hot loop, software-pipelined: group g's v-dots are emitted after
        # group g+1's energy matmuls so the PE never waits on the relu engines.
        groups = []
        for j in range(SLOTS):
            for c0 in range(0, NP[j], CHUNK):
                cw = min(CHUNK, NP[j] - c0)
                for s in range(0, cw, GRP):
                    groups.append((j, c0, s, min(GRP, cw - s)))

        psc_tiles = {}
        for j in range(SLOTS):
            psc_t = ps_sc.tile([128, nt_out], f32, tag="psc")
            psc_tiles[j] = psc_t

        pending = []  # list of (j, en_tile, start_pos, width)

        def flush_pending():
            for pj, pen, ppos, pw in pending:
                for k in range(0, pw, 128):
                    kw = min(128, pw - k)
                    tidx = (ppos + k) // 128
                    nc.tensor.matmul(
                        out=psc_tiles[pj][:kw, tidx : tidx + 1],
                        lhsT=pen[:, k : k + kw],
                        rhs=vvec_sb,
                        start=True,
                        stop=True,
                    )
            pending.clear()

        softmax_after = {}  # group index -> slot to run softmax for
        gi_of_slot_last = {}
        for gi, (j, c0, s, sw) in enumerate(groups):
            gi_of_slot_last[j] = gi
        for j, gi in gi_of_slot_last.items():
            softmax_after[gi] = j

        for gi, (j, c0, s, sw) in enumerate(groups):
            et = echunks[(j, c0)]
            pe = ps_e.tile([H, GRP], f32, tag="pe")
            for m in range(0, sw, MMN):
                mw = min(MMN, sw - m)
                nc.tensor.matmul(
                    out=pe[:, m : m + mw],
                    lhsT=w2t_sb,
                    rhs=et[:, s + m : s + m + mw],
                    start=True,
                    stop=True,
                )
            en = enpool.tile([H, GRP], dt_c, tag="en")
            if gi % 5 < 3:  # ~60% ScalarE / 40% VectorE
                nc.scalar.activation(
                    en[:, :sw], pe[:, :sw], AF.Relu, bias=hproj[:, j : j + 1]
                )
            else:
                nc.vector.tensor_scalar(
                    out=en[:, :sw],
                    in0=pe[:, :sw],
                    scalar1=hproj[:, j : j + 1],
                    scalar2=0.0,
                    op0=mybir.AluOpType.add,
                    op1=mybir.AluOpType.max,
                )
            if len(pending) >= 2:
                pj, pen, ppos, pw = pending.pop(0)
                for k in range(0, pw, 128):
                    kw = min(128, pw - k)
                    tidx = (ppos + k) // 128
                    nc.tensor.matmul(
                        out=psc_tiles[pj][:kw, tidx : tidx + 1],
                        lhsT=pen[:, k : k + kw],
                        rhs=vvec_sb,
                        start=True,
                        stop=True,
                    )
            pending.append((j, en, c0 + s, sw))
            if (gi - 2) in softmax_after:
                _softmax_slot(
                    nc, tc, softmax_after[gi - 2], NP, nt_out, psc_tiles, maskt_sb,
                    ident_sb, ones1, ones_col, smallp, outp, ps_h, ps_o, out, AF, f32,
                )
        flush_pending()
        ngr = len(groups)
        for g in (ngr - 2, ngr - 1):
            if g in softmax_after:
                _softmax_slot(
                    nc, tc, softmax_after[g], NP, nt_out, psc_tiles, maskt_sb,
                    ident_sb, ones1, ones_col, smallp, outp, ps_h, ps_o, out, AF, f32,
                )


def _softmax_slot(nc, tc, j, NP, nt_out, psc_tiles, maskt_sb, ident_sb, ones1,
                  ones_col, smallp, outp, ps_h, ps_o, out, AF, f32):
    """Fused masked softmax + transposed store for one slot."""
    if True:
        if True:
            nv = NP[j] // 128
            psc = psc_tiles[j]
            fmax = smallp.tile([128, 1], f32, tag="fmax")
            nc.vector.reduce_max(fmax[:], psc[:, :nv], axis=mybir.AxisListType.X)
            pmt = ps_h.tile([1, 128], f32, tag="ps_small")
            nc.tensor.transpose(pmt[:], fmax[:], ident_sb)
            negm = smallp.tile([1, 1], f32, tag="negm")
            nc.vector.tensor_reduce(
                negm[:], pmt[:], axis=mybir.AxisListType.X,
                op=mybir.AluOpType.max, negate=True,
            )
            pnb = ps_h.tile([128, 1], f32, tag="ps_small")
            nc.tensor.matmul(out=pnb[:], lhsT=ones1[:], rhs=negm[:], start=True, stop=True)
            negmb = smallp.tile([128, 1], f32, tag="negmb")
            nc.vector.tensor_copy(negmb[:], pnb[:])
            expm = smallp.tile([128, nt_out], f32, tag="expm")
            nc.scalar.activation(expm[:, :nv], psc[:, :nv], AF.Exp, bias=negmb[:])
            nc.vector.tensor_mul(expm[:, :nv], expm[:, :nv], maskt_sb[:, j, :nv])
            # sum chain (DVE/PE) runs in parallel with the output transpose (PE)
            psr = ps_h.tile([1, nt_out], f32, tag="ps_small")
            nc.tensor.matmul(
                out=psr[:, :nv], lhsT=ones_col[:], rhs=expm[:, :nv], start=True, stop=True
            )
            po = ps_o.tile([nt_out, 128], f32, tag="po")
            nc.tensor.transpose(po[:nv, :], expm[:, :nv], ident_sb)
            s11 = smallp.tile([1, 1], f32, tag="s11")
            nc.vector.reduce_sum(s11[:], psr[:, :nv], axis=mybir.AxisListType.X)
            nc.vector.reciprocal(s11[:], s11[:])
            prb = ps_h.tile([128, 1], f32, tag="ps_small")
            nc.tensor.matmul(out=prb[:], lhsT=ones1[:], rhs=s11[:], start=True, stop=True)
            recb = smallp.tile([128, 1], f32, tag="recb")
            nc.vector.tensor_copy(recb[:], prb[:])
            # fused normalize + PSUM drain
            ob = outp.tile([nt_out, 128], f32, tag="ob")
            nc.vector.tensor_scalar_mul(ob[:nv, :], po[:nv, :], recb[:nv])
            nc.sync.dma_start(out[j, :nv], ob[:nv, :])


def _make_inmaps(hidden, enc, ls, W_attn, b_attn, v, rows, NP, nt_out, dt_name):
    np_c = _np_dt(dt_name)
    f32 = np.float32
    ncf = H + 1 + SLOTS + 128 + SLOTS * nt_out
    c16 = np.empty((128, H + 1), np_c)
    c16[:, :H] = W_attn[:, H:].T.astype(np_c)  # w2t
    c16[:, H] = v.astype(np_c)  # vvec
    tgrid = (np.arange(nt_out)[None, :] * 128 + np.arange(128)[:, None])  # [128, nt]

    in_maps = []
    for i in range(NCORES):
        cf = np.empty((128, ncf), f32)
        cf[:, :H] = W_attn[:, :H].T  # w1t
        cf[:, H] = b_attn  # bvec
        o = H + 1 + SLOTS
        cf[:, o : o + 128] = np.eye(128, dtype=f32)  # ident
        m = {"constsf": cf, "consts16": c16}
        for j in range(SLOTS):
            b = rows[i][j]
            m[f"enc{j}"] = np.ascontiguousarray(enc[: NP[j], b, :].T).astype(np_c)
            cf[:, H + 1 + j] = hidden[b, :]  # hid
            cf[:, o + 128 + j * nt_out : o + 128 + (j + 1) * nt_out] = (
                tgrid < int(ls[b])
            ).astype(f32)  # maskt
        in_maps.append(m)
    return in_maps


def run(inputs, trace=False, **spmd_kwargs):
    hidden = np.asarray(inputs["hidden"], dtype=np.float32)
    enc = np.asarray(inputs["encoder_outputs"], dtype=np.float32)
    ls = np.asarray(inputs["len_seq"]).astype(np.int64)
    W_attn = np.asarray(inputs["W_attn"], dtype=np.float32)
    b_attn = np.asarray(inputs["b_attn"], dtype=np.float32)
    v = np.asarray(inputs["v"], dtype=np.float32)
    t_len = enc.shape[0]
    nt_out = t_len // 128

    rows, NP = _plan(ls, t_len)
    nc = bacc.Bacc("TRN2", target_bir_lowering=False, debug=False)
    _build(nc, NP, nt_out, COMPUTE_DTYPE)
    nc.compile()
    in_maps = _make_inmaps(hidden, enc, ls, W_attn, b_attn, v, rows, NP, nt_out,
                           COMPUTE_DTYPE)
    res = run_bass_kernel_spmd(
        nc, in_maps, core_ids=list(range(NCORES)), trace=trace, **spmd_kwargs
    )

    final = np.zeros((B, 1, t_len), dtype=np.float32)
    for i in range(NCORES):
        o = np.asarray(res.results[i]["out"], dtype=np.float32).reshape(SLOTS, t_len)
        for j in range(SLOTS):
            b = rows[i][j]
            ln = int(ls[b])
            final[b, 0, :ln] = o[j, :ln]
    return final, res


def kernel(**inputs):
    final, _ = run(inputs, trace=False)
    return final
